# revision 29
# baseline (speedup 1.0000x reference)
"""Trainium2 Bass kernel for a 16-head MHA block (B=1, S=2048, D=1024).

Reference semantics (PyTorch-style F.linear, W stored [out, in]):
    q/k/v = (x @ W.T).reshape(S, 16, 64) -> [h, s, d]
    scores = (q @ k.T) / 8, masked_fill(mask != 0, -1e-7)
    attn_scores = softmax(scores, -1)                       (output 2)
    out = scores @ v        <- PRE-softmax scores (faithful quirk)
    output = concat(out) @ W_o.T                            (output 1)

Sharding: 2 heads per core (tensor parallel). Each core computes its
head-slice projections from the full (host-transposed) inputs, both
orientations of the masked scores, the softmax, and a partial W_o
product (row-parallel W_o); the host sums the 8 partials.

Numerics note: masked positions hold -1e-7. We instead zero them for the
`scores @ v` contraction (|delta| ~ 1e-7 * |v|, relative ~3e-7) and feed
exp(0)=1 instead of exp(-1e-7) to the softmax (relative 1.2e-7). Both are
far below fp32 roundoff of the surrounding 1024/2048-term contractions.
"""

import math
import os

import ml_dtypes
import numpy as np

import concourse.bass as bass
import concourse.mybir as mybir
import concourse.tile as tile
from concourse.bass_utils import run_bass_kernel_spmd
from concourse.masks import make_identity

F32 = mybir.dt.float32
F32R = mybir.dt.float32r
BF16 = mybir.dt.bfloat16
U8 = mybir.dt.uint8

S = 2048
D = 1024
N_HEADS = 16
HD = 64  # head dim
NCORES = 8
HPC = N_HEADS // NCORES  # heads per core = 2
DD = HPC * HD  # per-core projection width = 128
SCALE = 1.0 / math.sqrt(HD)

# Matmul operand dtype: float32r streams at bf16 rate (N>=256) on trn2.
# Set BASS_MM_F32=1 to fall back to exact (4x slower) fp32 matmuls.
_USE_F32R = os.environ.get("BASS_MM_F32", "0") != "1"
# All tensors feeding the PE are declared float32r end-to-end: walrus
# requires f32r matmul operands to be produced as f32r by their writer.
MMDT = F32R if _USE_F32R else F32


def build_nc(split_waits=True, loop_n=None):
    """loop_n: benchmark-only variant that wraps the whole kernel body in a
    hardware For_i loop, so one dispatch executes the kernel loop_n times
    (dispatch overhead through the axon tunnel is ~90 ms and would otherwise
    swamp the ~0.3 ms kernel)."""
    nc = bass.Bass("TRN2", target_bir_lowering=False, debug=False)

    qT = nc.dram_tensor("qT", [D, S], MMDT, kind="ExternalInput").ap()
    kT = nc.dram_tensor("kT", [D, S], MMDT, kind="ExternalInput").ap()
    vT = nc.dram_tensor("vT", [D, S], MMDT, kind="ExternalInput").ap()
    wqT = nc.dram_tensor("wqT", [D, DD], MMDT, kind="ExternalInput").ap()
    wkT = nc.dram_tensor("wkT", [D, DD], MMDT, kind="ExternalInput").ap()
    wvT = nc.dram_tensor("wvT", [D, DD], MMDT, kind="ExternalInput").ap()
    woT = nc.dram_tensor("woT", [DD, D], MMDT, kind="ExternalInput").ap()
    nm = nc.dram_tensor("nm", [S, S], U8, kind="ExternalInput").ap()
    nmT = nc.dram_tensor("nmT", [S, S], U8, kind="ExternalInput").ap()

    attn = nc.dram_tensor("attn", [HPC, S, S], F32, kind="ExternalOutput").ap()
    partial = nc.dram_tensor("partial", [S, D], F32, kind="ExternalOutput").ap()

    with tile.TileContext(nc) as tc:
        if loop_n:
            with tc.For_i(0, loop_n, 1):
                build_kernel(
                    tc, qT, kT, vT, wqT, wkT, wvT, woT, nm, nmT, attn, partial
                )
        else:
            build_kernel(tc, qT, kT, vT, wqT, wkT, wvT, woT, nm, nmT, attn, partial)
    if split_waits:
        # NB: the CoreSim race detector can't digest post-scheduling NoOps;
        # sim tests build with split_waits=False (the pass is sync-hoisting
        # only and does not change semantics).
        _split_matmul_waits(nc)
    return nc


def _split_matmul_waits(nc):
    """Walrus codegen limits sync-waits per lowered struct: 1 for (f32r)
    Matmult (the weight-load struct carries the wait), 2 for DMA pseudo
    structs. Hoist extra waits onto preceding NoOps on the same engine
    stream, keeping updates on the original instruction."""
    split_types = (
        mybir.InstMatmult,
        mybir.InstDMACopy,
        mybir.InstDMA,
        mybir.InstDmaTransposeAnt,
        mybir.InstTensorTensor,
        mybir.InstTensorScalarPtr,
        mybir.InstActivation,
        mybir.InstTensorReduce,
        mybir.InstTensorTensorReduce,
        mybir.InstTensorCopy,
        mybir.InstReciprocal,
        mybir.InstCopyPredicated,
        mybir.InstMemset,
        mybir.InstStreamTranspose,
        mybir.InstStreamShuffle,
        mybir.InstIota,
        mybir.InstTensorScalarAffineSelect,
        mybir.InstCustomDveAnt,
        mybir.InstPool,
        mybir.InstISA,
        mybir.InstDrain,
        mybir.InstMax,
        mybir.InstMaxIndex,
        mybir.InstNoOp,
    )
    budget = 1
    for f in nc.m.functions:
        for b in f.blocks:
            out = []
            for inst in b.instructions:
                if (
                    isinstance(inst, split_types)
                    and inst.sync_info is not None
                    and len(inst.sync_info.on_wait) > budget
                ):
                    waits = list(inst.sync_info.on_wait)
                    extra, keep = waits[:-budget], waits[-budget:]
                    for w in extra:
                        out.append(
                            mybir.InstNoOp(
                                name=f"I-{nc.next_id()}",
                                engine=inst.engine,
                                sync_info=mybir.SyncInfo(on_wait=[w], on_update=[]),
                                bass_nofuse=True,
                            )
                        )
                    inst.sync_info = mybir.SyncInfo(
                        on_wait=keep, on_update=list(inst.sync_info.on_update)
                    )
                out.append(inst)
            if len(out) != len(b.instructions):
                try:
                    b.instructions[:] = out
                except TypeError:
                    b.instructions = out


def build_kernel(tc, qT, kT, vT, wqT, wkT, wvT, woT, nm, nmT, attn, partial):
    nc = tc.nc
    EC = D // 128  # 8 contraction chunks for projections
    TC = S // 128  # 16 key/row chunks
    NBLK = S // 512  # 4 column blocks of 512

    with (
        tc.tile_pool(name="wpool", bufs=1) as wpool,
        tc.tile_pool(name="persist", bufs=1) as persist,
    ):
        # ---- weights ----
        # wqT dram [D, DD] viewed as [ec, p, j] -> sbuf [p, ec, j]
        wq_sb = wpool.tile([128, EC, DD], MMDT)
        wk_sb = wpool.tile([128, EC, DD], MMDT)
        wv_sb = wpool.tile([128, EC, DD], MMDT)
        for w_sb, w_dr in ((wq_sb, wqT), (wk_sb, wkT), (wv_sb, wvT)):
            nc.sync.dma_start(w_sb, w_dr.rearrange("(c p) j -> p c j", p=128))
        # W_o.T halves loaded to partition base 0 each (matmul dst/partition
        # constraints: everything lives at partition base 0, contraction over
        # the two 64-row halves accumulates in PSUM)
        wo0_sb = wpool.tile([64, D], MMDT)
        wo1_sb = wpool.tile([64, D], MMDT)
        nc.sync.dma_start(wo0_sb, woT[0:64, :])
        nc.sync.dma_start(wo1_sb, woT[64:128, :])
        ident = wpool.tile([128, 128], F32)
        make_identity(nc, ident)

        # ---- persistent per-core tensors ----
        qT_sb = persist.tile([128, S], MMDT)  # q_c.T  [dd, s]
        kT_sb = persist.tile([128, S], MMDT)  # k_c.T  [dd, s]
        vTs_sb = persist.tile([128, S], F32)  # v_c.T * 0.125  [dd, t]
        v_sb = persist.tile([128, TC, DD], MMDT)  # v_c * 0.125  [t, dd]
        outT0_sb = persist.tile([64, S], MMDT)  # head0 (scores@v).T * 0.125
        outT1_sb = persist.tile([64, S], MMDT)  # head1

        # ---- projections: x_c.T[dd, s] = sum_e wxT[e, dd].T @ xT[e, s] ----
        with (
            tc.tile_pool(name="projio", bufs=4) as projio,
            tc.tile_pool(name="projps", bufs=2, space="PSUM") as projps,
        ):
            for dst, scale_, w_sb, x_dr in (
                (qT_sb, 1.0, wq_sb, qT),
                (kT_sb, 1.0, wk_sb, kT),
                (vTs_sb, SCALE, wv_sb, vT),
            ):
                for sb in range(NBLK):
                    ps = projps.tile([128, 512], F32, tag="ps")
                    for ec in range(EC):
                        xt = projio.tile([128, 512], MMDT, tag="xin")
                        nc.sync.dma_start(
                            xt, x_dr[ec * 128 : (ec + 1) * 128, sb * 512 : (sb + 1) * 512]
                        )
                        nc.tensor.matmul(
                            ps,
                            lhsT=(w_sb[:, ec, :]),
                            rhs=(xt),
                            start=(ec == 0),
                            stop=(ec == EC - 1),
                        )
                    nc.scalar.mul(dst[:, sb * 512 : (sb + 1) * 512], ps, scale_)

            # v natural layout [t, dd] via PE transpose of vTs_sb tiles
            for t in range(TC):
                pst = projps.tile([128, 128], F32, tag="pst")
                nc.tensor.transpose(pst, vTs_sb[:, t * 128 : (t + 1) * 128], ident)
                nc.scalar.copy(v_sb[:, t, :], pst)

        # ---- attention ----
        with (
            tc.tile_pool(name="sps", bufs=3, space="PSUM") as sps,
            tc.tile_pool(name="pout", bufs=1, space="PSUM") as pout,
            tc.tile_pool(name="mpool", bufs=3) as mpool,
            tc.tile_pool(name="atpool", bufs=6) as atpool,
            tc.tile_pool(name="epool", bufs=3) as epool,
            tc.tile_pool(name="apool", bufs=3) as apool,
            tc.tile_pool(name="smpool", bufs=4) as smpool,
            tc.tile_pool(name="wosb", bufs=3) as wosb,
        ):
            for sb in range(NBLK):
                scol = slice(sb * 512, (sb + 1) * 512)

                # --- pass T: A.T[t, s_blk] both heads; outT accumulation ---
                # separate banks per head, both at partition base 0 (ISA
                # requires matmul dst partition base 0)
                po0 = pout.tile([64, 512], F32, tag="po0", bufs=1)
                po1 = pout.tile([64, 512], F32, tag="po1", bufs=1)
                for t in range(TC):
                    nmT_t = mpool.tile([128, 512], U8, tag="nmT")
                    nc.sync.dma_start(nmT_t, nmT[t * 128 : (t + 1) * 128, scol])
                    psT = sps.tile([128, 1024], F32, tag="psT", bufs=1)
                    for h in range(HPC):
                        hrow = slice(h * 64, (h + 1) * 64)
                        nc.tensor.matmul(
                            psT[:, h * 512 : (h + 1) * 512],
                            lhsT=(kT_sb[hrow, t * 128 : (t + 1) * 128]),
                            rhs=(qT_sb[hrow, scol]),
                            start=True,
                            stop=True,
                        )
                    at = atpool.tile([128, 2, 512], MMDT, tag="at")
                    nc.vector.tensor_mul(
                        at,
                        psT.rearrange("p (h c) -> p h c", h=2),
                        nmT_t.rearrange("p (o c) -> p o c", o=1).broadcast_to(
                            [128, 2, 512]
                        ),
                    )
                    for h, po in ((0, po0), (1, po1)):
                        nc.tensor.matmul(
                            po,
                            lhsT=(v_sb[:, t, h * 64 : (h + 1) * 64]),
                            rhs=(at[:, h, :]),
                            start=(t == 0),
                            stop=(t == TC - 1),
                        )
                nc.scalar.copy(outT0_sb[:, scol], po0)
                nc.scalar.copy(outT1_sb[:, scol], po1)

                # --- W_o partial rows for this block (overlaps next blocks):
                # partial[s, :] = outT0[:, s].T @ wo0 + outT1[:, s].T @ wo1
                for sc4 in range(4):
                    srow4 = slice(sb * 512 + sc4 * 128, sb * 512 + (sc4 + 1) * 128)
                    pr = wosb.tile([128, D], F32, tag="pr")
                    for jc in range(2):
                        jcol = slice(jc * 512, (jc + 1) * 512)
                        pw = pout.tile([128, 512], F32, tag=("po0" if jc == 0 else "po1"), bufs=1)
                        nc.tensor.matmul(
                            pw,
                            lhsT=(outT0_sb[:, srow4]),
                            rhs=(wo0_sb[:, jcol]),
                            start=True,
                            stop=False,
                        )
                        nc.tensor.matmul(
                            pw,
                            lhsT=(outT1_sb[:, srow4]),
                            rhs=(wo1_sb[:, jcol]),
                            start=False,
                            stop=True,
                        )
                        if jc == 0:
                            nc.scalar.copy(pr[:, jcol], pw)
                        else:
                            nc.vector.tensor_copy(pr[:, jcol], pw)
                    nc.sync.dma_start(partial[srow4, :], pr)

                # --- pass S: scores[s, t], softmax, attn out ---
                for sc in range(4):
                    srow = slice(sb * 512 + sc * 128, sb * 512 + (sc + 1) * 128)
                    nm_t = mpool.tile([128, S], U8, tag="nm")
                    nc.sync.dma_start(nm_t, nm[srow, :])
                    for h in range(HPC):
                        hrow = slice(h * 64, (h + 1) * 64)
                        et = epool.tile([128, S], F32, tag="et")
                        a2 = epool.tile([128, S], F32, tag="a2", bufs=2)
                        acc = smpool.tile([128, 1], F32, tag="acc")
                        for tb in range(2):
                            psS = sps.tile([128, 1024], F32, tag="psS", bufs=2)
                            for u in range(2):
                                tcol = slice(
                                    (tb * 2 + u) * 512, (tb * 2 + u + 1) * 512
                                )
                                nc.tensor.matmul(
                                    psS[:, u * 512 : (u + 1) * 512],
                                    lhsT=(qT_sb[hrow, srow]),
                                    rhs=(kT_sb[hrow, tcol]),
                                    start=True,
                                    stop=True,
                                )
                            nc.vector.tensor_mul(
                                a2[:, tb * 1024 : (tb + 1) * 1024],
                                psS,
                                nm_t[:, tb * 1024 : (tb + 1) * 1024],
                            )
                        nc.scalar.activation(
                            et,
                            a2,
                            mybir.ActivationFunctionType.Exp,
                            scale=SCALE,
                            accum_out=acc,
                        )
                        rec = smpool.tile([128, 1], F32, tag="rec")
                        nc.vector.reciprocal(rec, acc)
                        arow = apool.tile([128, S], F32, tag="arow")
                        nc.scalar.mul(arow, et, rec)
                        nc.sync.dma_start(attn[h, srow, :], arow)



_NC_CACHE = None


def _get_nc():
    global _NC_CACHE
    if _NC_CACHE is None:
        _NC_CACHE = build_nc()
    return _NC_CACHE


def make_in_maps(query, key, value, attn_mask, W_q, W_k, W_v, W_o):
    q = np.asarray(query, np.float32)[0]
    k = np.asarray(key, np.float32)[0]
    v = np.asarray(value, np.float32)[0]
    mask = np.asarray(attn_mask)[0]

    qT = np.ascontiguousarray(q.T)
    kT = np.ascontiguousarray(k.T)
    vT = np.ascontiguousarray(v.T)
    nm = np.ascontiguousarray((mask == 0)).astype(np.uint8)
    nmT = np.ascontiguousarray((mask.T == 0)).astype(np.uint8)

    W_q = np.asarray(W_q, np.float32)
    W_k = np.asarray(W_k, np.float32)
    W_v = np.asarray(W_v, np.float32)
    W_o = np.asarray(W_o, np.float32)

    in_maps = []
    for c in range(NCORES):
        sl = slice(c * DD, (c + 1) * DD)
        in_maps.append(
            {
                "qT": qT,
                "kT": kT,
                "vT": vT,
                "wqT": np.ascontiguousarray(W_q[sl, :].T),
                "wkT": np.ascontiguousarray(W_k[sl, :].T),
                "wvT": np.ascontiguousarray(W_v[sl, :].T),
                "woT": np.ascontiguousarray(W_o[:, sl].T),
                "nm": nm,
                "nmT": nmT,
            }
        )
    return in_maps


def kernel(query, key, value, attn_mask, W_q, W_k, W_v, W_o, _results_hook=None):
    nc = _get_nc()
    in_maps = make_in_maps(query, key, value, attn_mask, W_q, W_k, W_v, W_o)
    res = run_bass_kernel_spmd(nc, in_maps, core_ids=list(range(NCORES)))
    if _results_hook is not None:
        _results_hook(res)
    results = res.results
    attn_full = np.concatenate([r["attn"] for r in results], axis=0)[None]
    output = np.sum([r["partial"] for r in results], axis=0, dtype=np.float32)[None]
    return output.astype(np.float32), attn_full.astype(np.float32)
